# revision 13
# baseline (speedup 1.0000x reference)
"""StyleGAN2 modulated conv_transpose (stride=1, pad=1) for Trainium2.

Strategy (data-parallel over batch, 2 samples per core on 8 cores):
  conv_transpose2d(x, w_mod) with per-sample modulated+demodulated weights
  factors exactly as
      out_b[o] = dinv_b[o] * conv2d(s_b (.) x_b, W*HE)[o] + GAIN*bias[o]
  with dinv computed exactly on the host.  The conv itself runs as a
  1D Winograd F(2,3) along W (1.5x fewer MACs than direct):
      y[h, 2j+r] = sum_c AT[r,c] M_c[h,j]
      M_c = sum_a (G W)_[a,c]^T  @  V_c[rows h+a]     (H stays direct)
      V_c[h,j]   = sum_q BT[c,q] xpad[h, 2j+q-1]
  Host (free): style-scale x, Winograd+HE weight transform, demod dinv,
  GAIN*bias; everything cast to bf16 for the PE (f32 PSUM accumulate).
  Device: DVE input transform + inverse combine, PE matmuls (bf16, FWL),
  Act PSUM evacuation + final scale/bias.  DMAs are few and fat
  (contiguous per-partition rows): inputs serial on the SP HWDGE ring in
  PE-consumption order, outputs/consts on SWDGE.
"""

from contextlib import ExitStack

import numpy as np
import ml_dtypes

import concourse.bass as bass
from concourse import bacc
import concourse.mybir as mybir
import concourse.tile as tile
from concourse.bass_utils import run_bass_kernel_spmd

TRACE = False
TRACE_KW = {}
LAST_RESULT = None
MODE = "bf16"

B, C, H, W, KK = 16, 512, 32, 32, 3
NCORES, BPC = 8, B // 8
KT = C // 128  # k-tiles over in-channels
MT = C // 128  # m-tiles over out-channels
NC_ = 4        # Winograd F(2,3) components along W
NT_ = W // 2   # output tiles along W (2 cols per tile)
GAIN = 1.4142135623730951
HE = GAIN / float(C * KK * KK) ** 0.5
EPS = 1e-8

F32 = mybir.dt.float32
BF16 = mybir.dt.bfloat16

# F(2,3) correlation form: V0 = x[2j-1]-x[2j+1], V1 = x[2j]+x[2j+1],
# V2 = x[2j+1]-x[2j], V3 = x[2j]-x[2j+2];  yE = M0+M1+M2, yO = M1-M2-M3
_G = np.array(
    [[1, 0, 0], [0.5, 0.5, 0.5], [0.5, -0.5, 0.5], [0, 0, 1]], np.float64
)

# tap order per component: center (full-height, start=True) first
_A_ORDER = [1, 0, 2]


def _build():
    nc = bacc.Bacc("TRN2", target_bir_lowering=False, num_swdge_queues=4)
    xs_d = nc.declare_dram_parameter("xs", [128, BPC * KT * H * W], BF16, isOutput=False)
    w_d = nc.declare_dram_parameter("wt", [128, NC_ * KK * KT * C], BF16, isOutput=False)
    dv_d = nc.declare_dram_parameter("dinv", [128, MT * BPC], F32, isOutput=False)
    gb_d = nc.declare_dram_parameter("gb", [128, MT], F32, isOutput=False)
    out_d = nc.declare_dram_parameter("out", [MT, BPC, 128, H * W], F32, isOutput=True)

    with tile.TileContext(nc) as tc, ExitStack() as ctx:
        singles = ctx.enter_context(tc.tile_pool(name="singles", bufs=1))
        mspool = ctx.enter_context(tc.tile_pool(name="mspool", bufs=3))
        tmps = ctx.enter_context(tc.tile_pool(name="tmps", bufs=3))
        osbp = ctx.enter_context(tc.tile_pool(name="osbp", bufs=3))
        cpsum = ctx.enter_context(tc.tile_pool(name="cpsum", bufs=8, space="PSUM"))

        # ---- bulk input DMAs: few and fat, contiguous per-partition rows.
        # The SWDGE queues start draining earliest (~2.5us) -> first x
        # halves go there; the weight stream runs serially on the SP HWDGE
        # ring in PE-consumption order (c0 split in k-halves so the first
        # matmuls start sooner).
        xt = {}
        xt[0] = singles.tile([128, KT, H, W], BF16, tag="xt_0", name="xt0")
        half = KT // 2 * H * W
        nc.gpsimd.dma_start(
            out=xt[0][:, : KT // 2],
            in_=xs_d[:, :half].rearrange("p (k h w) -> p k h w", k=KT // 2, h=H),
        )
        nc.gpsimd.dma_start(
            out=xt[0][:, KT // 2 :],
            in_=xs_d[:, half : 2 * half].rearrange(
                "p (k h w) -> p k h w", k=KT // 2, h=H
            ),
        )
        dv_t = singles.tile([128, MT * BPC], F32, tag="dv_t")
        nc.gpsimd.dma_start(out=dv_t, in_=dv_d[:])
        gb_t = singles.tile([128, MT], F32, tag="gb_t")
        nc.gpsimd.dma_start(out=gb_t, in_=gb_d[:])

        # weight layout [p, c, k, a, o] so k-half slabs are contiguous
        w_mm = singles.tile([128, NC_, KT, KK, C], BF16, tag="w_mm")

        def w_dma(c, klo, khi):
            base = (c * KT + klo) * KK * C
            nc.sync.dma_start(
                out=w_mm[:, c, klo:khi],
                in_=w_d[:, base : base + (khi - klo) * KK * C].rearrange(
                    "p (k a o) -> p k a o", k=khi - klo, a=KK
                ),
            )

        w_dma(0, 0, 2)
        w_dma(0, 2, 4)
        for c in range(1, NC_):
            w_dma(c, 0, KT)
        # second sample's x last on the SP ring (needed ~35us in)
        xt[1] = singles.tile([128, KT, H, W], BF16, tag="xt_1", name="xt1")
        nc.sync.dma_start(
            out=xt[1],
            in_=xs_d[:, KT * H * W :].rearrange("p (k h w) -> p k h w", k=KT, h=H),
        )

        # ---- PE warmup: dummy matmuls on zeros release the HAM clock gate
        # (PE runs 1.2 GHz cold, 2.4 warm) while the DMAs land.
        wz_t = singles.tile([128, 512], BF16, tag="wz_t")
        nc.vector.memset(wz_t, 0.0)
        wps = cpsum.tile([128, H, NT_], F32, tag="cps", name="wps")
        for _ in range(16):
            nc.tensor.matmul(
                wps.rearrange("p h j -> p (h j)"),
                wz_t[:, :128],
                wz_t[:, :512],
                start=True,
                stop=True,
            )

        # ---- input transform: V_c rows 0..33 = x rows -1..32 (borders
        # zero) so every matmul is full-height.  Column edges (x[-1], x[32])
        # are pure padding -> two tiny fixup ops instead of a padded copy.
        V = {}
        for s in range(BPC):
            for k in range(KT):
                vt = singles.tile([128, NC_, H + 2, NT_], BF16, tag=f"v_{k}_{s}")
                vborder = bass.AP(
                    tensor=vt.tensor,
                    offset=vt.offset,
                    ap=[vt.ap[0], [(H + 2) * NT_, NC_], [(H + 1) * NT_, 2], [1, NT_]],
                )
                nc.vector.memset(vborder, 0.0)
                V[k, s] = vt
            # c-major, k-inner: component c of all k completes early
            for c in range(NC_):
                for k in range(KT):
                    vt = V[k, s]

                    def xv(col0, nj):
                        # [128, H, nj] view of x at cols col0, col0+2, ...
                        base = xt[s][:, k]
                        return bass.AP(
                            tensor=base.tensor,
                            offset=base.offset + col0,
                            ap=[base.ap[0], [W, H], [2, nj]],
                        )

                    if c == 0:
                        # j=0: V0 = x[-1]-x[1] = -x[1]
                        nc.vector.tensor_scalar_mul(
                            vt[:, 0, 1 : H + 1, 0:1], xv(1, 1), -1.0
                        )
                        nc.vector.tensor_sub(
                            vt[:, 0, 1 : H + 1, 1:], xv(1, NT_ - 1), xv(3, NT_ - 1)
                        )
                    elif c == 1:
                        nc.vector.tensor_add(
                            vt[:, 1, 1 : H + 1, :], xv(0, NT_), xv(1, NT_)
                        )
                    elif c == 2:
                        nc.vector.tensor_sub(
                            vt[:, 2, 1 : H + 1, :], xv(1, NT_), xv(0, NT_)
                        )
                    else:
                        nc.vector.tensor_sub(
                            vt[:, 3, 1 : H + 1, : NT_ - 1],
                            xv(0, NT_ - 1),
                            xv(2, NT_ - 1),
                        )
                        # j=15: V3 = x[30]-x[32] = x[30]
                        nc.vector.tensor_copy(
                            vt[:, 3, 1 : H + 1, NT_ - 1 :], xv(30, 1)
                        )

        # ---- conv: groups (m, s) of 4 PSUM banks each; two groups in
        # flight across the 8-bank pool.  The first two groups are
        # c-interleaved so the PE tracks the weight-stream arrival order.
        out_engines = [nc.sync, nc.sync]
        oi = 0

        # (a, k) emission order: k-halves first so c0's first matmuls only
        # need the first half-slab of weights/x
        _AK = [(a, k) for kh in range(2) for a in _A_ORDER for k in (2 * kh, 2 * kh + 1)]

        def mm_group_part(m, s, cps, c):
            for i, (a, k) in enumerate(_AK):
                # V row index = out row h + a; skip rows whose x tap is
                # pure padding (a=0: h=0, a=2: h=31)
                h_lo = 1 if a == 0 else 0
                h_hi = H - 1 if a == 2 else H
                nc.tensor.matmul(
                    cps[c][:, h_lo:h_hi, :],
                    w_mm[:, c, k, a, m * 128 : (m + 1) * 128],
                    V[k, s][:, c, h_lo + a : h_hi + a, :],
                    start=(i == 0),
                    stop=(i == len(_AK) - 1),
                )

        def drain_group(m, s, cps):
            nonlocal oi
            m1 = mspool.tile([128, H, NT_], F32, tag="m1")
            nc.scalar.copy(m1, cps[1])
            m2 = mspool.tile([128, H, NT_], F32, tag="m2")
            nc.scalar.copy(m2, cps[2])
            t_e = tmps.tile([128, H, NT_], F32, tag="t_e")
            nc.vector.tensor_add(t_e, m1, m2)
            t_o = tmps.tile([128, H, NT_], F32, tag="t_o")
            nc.vector.tensor_sub(t_o, m1, m2)
            osb = osbp.tile([128, H, W], F32, tag="osb")
            oeven = bass.AP(
                tensor=osb.tensor,
                offset=osb.offset,
                ap=[osb.ap[0], [W, H], [2, NT_]],
            )
            oodd = bass.AP(
                tensor=osb.tensor,
                offset=osb.offset + 1,
                ap=[osb.ap[0], [W, H], [2, NT_]],
            )
            nc.vector.tensor_add(oeven, t_e, cps[0])
            nc.vector.tensor_sub(oodd, t_o, cps[3])
            osb2 = osbp.tile([128, H * W], F32, tag="osb2")
            nc.scalar.activation(
                osb2,
                osb.rearrange("p h w -> p (h w)"),
                mybir.ActivationFunctionType.Identity,
                bias=gb_t[:, m : m + 1],
                scale=dv_t[:, m * BPC + s : m * BPC + s + 1],
            )
            out_engines[oi % 2].dma_start(out=out_d[m, s], in_=osb2)
            oi += 1

        # first pair (m0, m1) of sample 0: c-interleaved across both groups
        cps_a = [cpsum.tile([128, H, NT_], F32, tag="cps", name="cps") for _ in range(NC_)]
        cps_b = [cpsum.tile([128, H, NT_], F32, tag="cps", name="cps") for _ in range(NC_)]
        for c in range(NC_):
            mm_group_part(0, 0, cps_a, c)
            mm_group_part(1, 0, cps_b, c)
        drain_group(0, 0, cps_a)
        drain_group(1, 0, cps_b)
        # remaining groups sequential
        for s in range(BPC):
            for m in range(MT):
                if s == 0 and m < 2:
                    continue
                cps = [cpsum.tile([128, H, NT_], F32, tag="cps", name="cps") for _ in range(NC_)]
                for c in range(NC_):
                    mm_group_part(m, s, cps, c)
                drain_group(m, s, cps)
    nc.finalize()
    return nc


def kernel(inp, style, weight, bias):
    global LAST_RESULT
    inp = np.asarray(inp, np.float32)
    style = np.asarray(style, np.float32)
    weight = np.asarray(weight, np.float32)
    bias = np.asarray(bias, np.float32)

    # ---- host prep (exact, cheap) ----
    # conv kernel (o,i,a,q) = flipped conv_transpose kernel, HE folded
    Wk = np.flip(weight, axis=(2, 3)).transpose(1, 0, 2, 3).astype(np.float64) * HE
    # Winograd weight transform along W-taps: Wh[a,c,i,o] -> [p,c,k,a,o]
    Wh = np.einsum("cq,oiaq->acio", _G, Wk)
    w_host = np.ascontiguousarray(
        Wh.reshape(KK, NC_, KT, 128, C).transpose(3, 1, 2, 0, 4)
    ).astype(ml_dtypes.bfloat16).reshape(128, NC_ * KK * KT * C)

    # demod denominators (exact)
    R = np.sum(weight.astype(np.float64) ** 2, axis=(2, 3))  # (in, out)
    d2 = HE * HE * (style.astype(np.float64) ** 2) @ R + EPS  # (b, out)
    dinv = (GAIN / np.sqrt(d2)).astype(np.float32)  # (b, out)
    gbias = (GAIN * bias).astype(np.float32)  # (out,)

    # style-scaled input, bf16, host layout [p, (s, k), hw]
    xs = (inp * style[:, :, None, None]).reshape(B, KT, 128, H * W)
    xs = xs.astype(ml_dtypes.bfloat16)

    nc = _build()
    in_maps = []
    for cc in range(NCORES):
        sl = slice(cc * BPC, (cc + 1) * BPC)
        xs_c = np.ascontiguousarray(
            xs[sl].transpose(2, 0, 1, 3)
        ).reshape(128, BPC * KT * H * W)
        dv_c = np.ascontiguousarray(
            dinv[sl].reshape(BPC, MT, 128).transpose(2, 1, 0)
        ).reshape(128, MT * BPC)
        gb_c = np.ascontiguousarray(gbias.reshape(MT, 128).T)
        in_maps.append({"xs": xs_c, "wt": w_host, "dinv": dv_c, "gb": gb_c})
    res = run_bass_kernel_spmd(
        nc, in_maps, list(range(NCORES)), trace=TRACE, **TRACE_KW
    )
    LAST_RESULT = res
    outs = []
    for cc in range(NCORES):
        o = res.results[cc]["out"]  # [MT, BPC, 128, HW]
        outs.append(np.asarray(o).transpose(1, 0, 2, 3).reshape(BPC, C, H, W))
    return np.concatenate(outs, axis=0)


# revision 17
# speedup vs baseline: 1.0612x; 1.0612x over previous
"""StyleGAN2 modulated conv_transpose (stride=1, pad=1) for Trainium2.

Strategy (data-parallel over batch, 2 samples per core on 8 cores):
  conv_transpose2d(x, w_mod) with per-sample modulated+demodulated weights
  factors exactly as
      out_b[o] = dinv_b[o] * conv2d(s_b (.) x_b, W*HE)[o] + GAIN*bias[o]
  with dinv computed exactly on the host.  The conv itself runs as a
  1D Winograd F(2,3) along W (1.5x fewer MACs than direct):
      y[h, 2j+r] = sum_c AT[r,c] M_c[h,j]
      M_c = sum_a (G W)_[a,c]^T  @  V_c[rows h+a]     (H stays direct)
      V_c[h,j]   = sum_q BT[c,q] xpad[h, 2j+q-1]
  Host (free): style-scale x, Winograd+HE weight transform, demod dinv,
  GAIN*bias; everything cast to bf16 for the PE (f32 PSUM accumulate).
  Device: DVE input transform + inverse combine, PE matmuls (bf16, FWL),
  Act PSUM evacuation + final scale/bias.  DMAs are few and fat
  (contiguous per-partition rows): inputs serial on the SP HWDGE ring in
  PE-consumption order, outputs/consts on SWDGE.
"""

from contextlib import ExitStack

import numpy as np
import ml_dtypes

import concourse.bass as bass
from concourse import bacc
import concourse.mybir as mybir
import concourse.tile as tile
from concourse.bass_utils import run_bass_kernel_spmd

TRACE = False
TRACE_KW = {}
LAST_RESULT = None
MODE = "bf16"

B, C, H, W, KK = 16, 512, 32, 32, 3
NCORES, BPC = 8, B // 8
KT = C // 128  # k-tiles over in-channels
MT = C // 128  # m-tiles over out-channels
NC_ = 4        # Winograd F(2,3) components along W
NT_ = W // 2   # output tiles along W (2 cols per tile)
GAIN = 1.4142135623730951
HE = GAIN / float(C * KK * KK) ** 0.5
EPS = 1e-8

F32 = mybir.dt.float32
BF16 = mybir.dt.bfloat16

# F(2,3) correlation form: V0 = x[2j-1]-x[2j+1], V1 = x[2j]+x[2j+1],
# V2 = x[2j+1]-x[2j], V3 = x[2j]-x[2j+2];  yE = M0+M1+M2, yO = M1-M2-M3
_G = np.array(
    [[1, 0, 0], [0.5, 0.5, 0.5], [0.5, -0.5, 0.5], [0, 0, 1]], np.float64
)

# tap order per component: center (full-height, start=True) first
_A_ORDER = [1, 0, 2]


def _build():
    nc = bacc.Bacc("TRN2", target_bir_lowering=False, num_swdge_queues=4)
    xs_d = nc.declare_dram_parameter("xs", [128, BPC * KT * H * W], BF16, isOutput=False)
    w_d = nc.declare_dram_parameter("wt", [128, NC_ * KK * KT * C], BF16, isOutput=False)
    dv_d = nc.declare_dram_parameter("dinv", [128, MT * BPC], F32, isOutput=False)
    gb_d = nc.declare_dram_parameter("gb", [128, MT], F32, isOutput=False)
    out_d = nc.declare_dram_parameter("out", [MT, BPC, 128, H * W], F32, isOutput=True)

    with tile.TileContext(nc) as tc, ExitStack() as ctx:
        singles = ctx.enter_context(tc.tile_pool(name="singles", bufs=1))
        mspool = ctx.enter_context(tc.tile_pool(name="mspool", bufs=3))
        tmps = ctx.enter_context(tc.tile_pool(name="tmps", bufs=3))
        osbp = ctx.enter_context(tc.tile_pool(name="osbp", bufs=3))
        cpsum = ctx.enter_context(tc.tile_pool(name="cpsum", bufs=8, space="PSUM"))

        # ---- bulk input DMAs: few and fat, contiguous per-partition rows.
        # The SWDGE queues start draining earliest (~2.5us) -> first x
        # halves go there; the weight stream runs serially on the SP HWDGE
        # ring in PE-consumption order (c0 split in k-halves so the first
        # matmuls start sooner).
        xt = {}
        xt[0] = singles.tile([128, KT, H, W], BF16, tag="xt_0", name="xt0")
        half = KT // 2 * H * W
        nc.sync.dma_start(
            out=xt[0][:, : KT // 2],
            in_=xs_d[:, :half].rearrange("p (k h w) -> p k h w", k=KT // 2, h=H),
        )
        nc.sync.dma_start(
            out=xt[0][:, KT // 2 :],
            in_=xs_d[:, half : 2 * half].rearrange(
                "p (k h w) -> p k h w", k=KT // 2, h=H
            ),
        )
        dv_t = singles.tile([128, MT * BPC], F32, tag="dv_t")
        nc.gpsimd.dma_start(out=dv_t, in_=dv_d[:])
        gb_t = singles.tile([128, MT], F32, tag="gb_t")
        nc.gpsimd.dma_start(out=gb_t, in_=gb_d[:])

        # weight layout [p, c, k, a, o] so k-half slabs are contiguous
        w_mm = singles.tile([128, NC_, KT, KK, C], BF16, tag="w_mm")

        def w_dma(c, klo, khi):
            base = (c * KT + klo) * KK * C
            nc.sync.dma_start(
                out=w_mm[:, c, klo:khi],
                in_=w_d[:, base : base + (khi - klo) * KK * C].rearrange(
                    "p (k a o) -> p k a o", k=khi - klo, a=KK
                ),
            )

        w_dma(0, 0, 2)
        w_dma(0, 2, 4)
        for c in range(1, NC_):
            w_dma(c, 0, KT)
        # second sample's x last on the SP ring (needed ~35us in)
        xt[1] = singles.tile([128, KT, H, W], BF16, tag="xt_1", name="xt1")
        nc.sync.dma_start(
            out=xt[1],
            in_=xs_d[:, KT * H * W :].rearrange("p (k h w) -> p k h w", k=KT, h=H),
        )

        # ---- PE warmup: dummy matmuls on zeros release the HAM clock gate
        # (PE runs 1.2 GHz cold, 2.4 warm) while the DMAs land.
        wz_t = singles.tile([128, 512], BF16, tag="wz_t")
        nc.vector.memset(wz_t, 0.0)
        wps = cpsum.tile([128, H, NT_], F32, tag="cps", name="wps")
        for _ in range(10):
            nc.tensor.matmul(
                wps.rearrange("p h j -> p (h j)"),
                wz_t[:, :128],
                wz_t[:, :512],
                start=True,
                stop=True,
            )

        # ---- input transform: V_c rows 0..33 = x rows -1..32 (borders
        # zero) so every matmul is full-height.  Column edges (x[-1], x[32])
        # are pure padding -> two tiny fixup ops instead of a padded copy.
        V = {}
        for s in range(BPC):
            for k in range(KT):
                vt = singles.tile([128, NC_, H + 2, NT_], BF16, tag=f"v_{k}_{s}")
                vborder = bass.AP(
                    tensor=vt.tensor,
                    offset=vt.offset,
                    ap=[vt.ap[0], [(H + 2) * NT_, NC_], [(H + 1) * NT_, 2], [1, NT_]],
                )
                nc.vector.memset(vborder, 0.0)
                V[k, s] = vt
            # c-major, k-inner: component c of all k completes early
            for c in range(NC_):
                for k in range(KT):
                    vt = V[k, s]

                    def xv(col0, nj):
                        # [128, H, nj] view of x at cols col0, col0+2, ...
                        base = xt[s][:, k]
                        return bass.AP(
                            tensor=base.tensor,
                            offset=base.offset + col0,
                            ap=[base.ap[0], [W, H], [2, nj]],
                        )

                    if c == 0:
                        # j=0: V0 = x[-1]-x[1] = -x[1]
                        nc.vector.tensor_scalar_mul(
                            vt[:, 0, 1 : H + 1, 0:1], xv(1, 1), -1.0
                        )
                        nc.vector.tensor_sub(
                            vt[:, 0, 1 : H + 1, 1:], xv(1, NT_ - 1), xv(3, NT_ - 1)
                        )
                    elif c == 1:
                        nc.vector.tensor_add(
                            vt[:, 1, 1 : H + 1, :], xv(0, NT_), xv(1, NT_)
                        )
                    elif c == 2:
                        nc.vector.tensor_sub(
                            vt[:, 2, 1 : H + 1, :], xv(1, NT_), xv(0, NT_)
                        )
                    else:
                        nc.vector.tensor_sub(
                            vt[:, 3, 1 : H + 1, : NT_ - 1],
                            xv(0, NT_ - 1),
                            xv(2, NT_ - 1),
                        )
                        # j=15: V3 = x[30]-x[32] = x[30]
                        nc.vector.tensor_copy(
                            vt[:, 3, 1 : H + 1, NT_ - 1 :], xv(30, 1)
                        )

        # ---- conv: groups (m, s) of 4 PSUM banks each; two groups in
        # flight across the 8-bank pool.  The first two groups are
        # c-interleaved so the PE tracks the weight-stream arrival order.
        out_engines = [nc.sync, nc.sync]
        oi = 0

        # (a, k) emission order: k-halves first so c0's first matmuls only
        # need the first half-slab of weights/x
        _AK = [(a, k) for kh in range(2) for a in _A_ORDER for k in (2 * kh, 2 * kh + 1)]

        def mm_group_part(m, s, cps, c, h0=0, h1=H):
            for i, (a, k) in enumerate(_AK):
                # V row index = out row h + a; skip rows whose x tap is
                # pure padding (a=0: h=0, a=2: h=31)
                h_lo = max(h0, 1 if a == 0 else 0)
                h_hi = min(h1, H - 1 if a == 2 else H)
                nc.tensor.matmul(
                    cps[c][:, h_lo - h0 : h_hi - h0, :],
                    w_mm[:, c, k, a, m * 128 : (m + 1) * 128],
                    V[k, s][:, c, h_lo + a : h_hi + a, :],
                    start=(i == 0),
                    stop=(i == len(_AK) - 1),
                )

        def drain_group(m, s, cps, h0=0, h1=H):
            nonlocal oi
            hh = h1 - h0
            m1 = mspool.tile([128, hh, NT_], F32, tag="m1", name="m1")
            nc.scalar.copy(m1, cps[1])
            m2 = mspool.tile([128, hh, NT_], F32, tag="m2", name="m2")
            nc.scalar.copy(m2, cps[2])
            t_e = tmps.tile([128, hh, NT_], F32, tag="t_e", name="t_e")
            nc.vector.tensor_add(t_e, m1, m2)
            t_o = tmps.tile([128, hh, NT_], F32, tag="t_o", name="t_o")
            nc.vector.tensor_sub(t_o, m1, m2)
            osb = osbp.tile([128, hh, W], F32, tag="osb", name="osb")
            oeven = bass.AP(
                tensor=osb.tensor,
                offset=osb.offset,
                ap=[osb.ap[0], [W, hh], [2, NT_]],
            )
            oodd = bass.AP(
                tensor=osb.tensor,
                offset=osb.offset + 1,
                ap=[osb.ap[0], [W, hh], [2, NT_]],
            )
            nc.vector.tensor_add(oeven, t_e, cps[0])
            nc.vector.tensor_sub(oodd, t_o, cps[3])
            osb2 = osbp.tile([128, hh * W], F32, tag="osb2", name="osb2")
            nc.scalar.activation(
                osb2,
                osb.rearrange("p h w -> p (h w)"),
                mybir.ActivationFunctionType.Identity,
                bias=gb_t[:, m : m + 1],
                scale=dv_t[:, m * BPC + s : m * BPC + s + 1],
            )
            out_engines[oi % 2].dma_start(
                out=out_d[m, s][:, h0 * W : h1 * W], in_=osb2
            )
            oi += 1

        # first pair (m0, m1) of sample 0: c-interleaved across both groups
        cps_a = [cpsum.tile([128, H, NT_], F32, tag="cps", name="cps") for _ in range(NC_)]
        cps_b = [cpsum.tile([128, H, NT_], F32, tag="cps", name="cps") for _ in range(NC_)]
        for c in range(NC_):
            mm_group_part(0, 0, cps_a, c)
            mm_group_part(1, 0, cps_b, c)
        drain_group(0, 0, cps_a)
        drain_group(1, 0, cps_b)
        # remaining groups sequential; last group h-split so its drain
        # pipeline overlaps the final matmuls instead of trailing them
        for s in range(BPC):
            for m in range(MT):
                if s == 0 and m < 2:
                    continue
                if s == BPC - 1 and m == MT - 1:
                    for h0 in (0, H // 2):
                        h1 = h0 + H // 2
                        cps = [
                            cpsum.tile(
                                [128, H // 2, NT_], F32, tag="cps", name="cps"
                            )
                            for _ in range(NC_)
                        ]
                        for c in range(NC_):
                            mm_group_part(m, s, cps, c, h0, h1)
                        drain_group(m, s, cps, h0, h1)
                else:
                    cps = [
                        cpsum.tile([128, H, NT_], F32, tag="cps", name="cps")
                        for _ in range(NC_)
                    ]
                    for c in range(NC_):
                        mm_group_part(m, s, cps, c)
                    drain_group(m, s, cps)
    nc.finalize()
    return nc


def kernel(inp, style, weight, bias):
    global LAST_RESULT
    inp = np.asarray(inp, np.float32)
    style = np.asarray(style, np.float32)
    weight = np.asarray(weight, np.float32)
    bias = np.asarray(bias, np.float32)

    # ---- host prep (exact, cheap) ----
    # conv kernel (o,i,a,q) = flipped conv_transpose kernel, HE folded
    Wk = np.flip(weight, axis=(2, 3)).transpose(1, 0, 2, 3).astype(np.float64) * HE
    # Winograd weight transform along W-taps: Wh[a,c,i,o] -> [p,c,k,a,o]
    Wh = np.einsum("cq,oiaq->acio", _G, Wk)
    w_host = np.ascontiguousarray(
        Wh.reshape(KK, NC_, KT, 128, C).transpose(3, 1, 2, 0, 4)
    ).astype(ml_dtypes.bfloat16).reshape(128, NC_ * KK * KT * C)

    # demod denominators (exact)
    R = np.sum(weight.astype(np.float64) ** 2, axis=(2, 3))  # (in, out)
    d2 = HE * HE * (style.astype(np.float64) ** 2) @ R + EPS  # (b, out)
    dinv = (GAIN / np.sqrt(d2)).astype(np.float32)  # (b, out)
    gbias = (GAIN * bias).astype(np.float32)  # (out,)

    # style-scaled input, bf16, host layout [p, (s, k), hw]
    xs = (inp * style[:, :, None, None]).reshape(B, KT, 128, H * W)
    xs = xs.astype(ml_dtypes.bfloat16)

    nc = _build()
    in_maps = []
    for cc in range(NCORES):
        sl = slice(cc * BPC, (cc + 1) * BPC)
        xs_c = np.ascontiguousarray(
            xs[sl].transpose(2, 0, 1, 3)
        ).reshape(128, BPC * KT * H * W)
        dv_c = np.ascontiguousarray(
            dinv[sl].reshape(BPC, MT, 128).transpose(2, 1, 0)
        ).reshape(128, MT * BPC)
        gb_c = np.ascontiguousarray(gbias.reshape(MT, 128).T)
        in_maps.append({"xs": xs_c, "wt": w_host, "dinv": dv_c, "gb": gb_c})
    res = run_bass_kernel_spmd(
        nc, in_maps, list(range(NCORES)), trace=TRACE, **TRACE_KW
    )
    LAST_RESULT = res
    outs = []
    for cc in range(NCORES):
        o = res.results[cc]["out"]  # [MT, BPC, 128, HW]
        outs.append(np.asarray(o).transpose(1, 0, 2, 3).reshape(BPC, C, H, W))
    return np.concatenate(outs, axis=0)
